# revision 54
# baseline (speedup 1.0000x reference)
"""Multi-head attention Bass/Tile kernel for 8 TRN2 NeuronCores.

Problem: nn_MultiHeadAttention (B=4, T1=T2=2048, d_model=256, d_key=32, H=8,
per-head value dim = d_model).  Reference math (no score scaling, no mask):

    k = key   @ WK^T + bk           [B, T1, 256]   (head h -> cols 32h..32h+32)
    q = query @ WQ^T + bq           [B, T2, 256]
    v = value @ WV^T + bv           [B, T1, 2048]  (head h -> cols 256h..256h+256)
    scores_h = k_h q_h^T            [T1, T2]
    attn = softmax over T1 (keys)
    emb_h = attn^T v_h              [T2, 256]
    out = emb' @ WO^T + bo          emb' channel c = d*8 + h (d outer, h inner)

Sharding: core c handles (batch b = c//2, query half qs = c%2) -> each core
computes the full output slice out[b, qs*1024:(qs+1)*1024, :].  No collectives.

Host-side marshaling (layout only, plus the same bf16 rounding the device
cast would do): activations and weights are shipped pre-transposed into the
channel-major layouts the PE consumes (keyT, qryT, valT, wvT, wkT, wqT, and
WO column-permuted head-outer), as bf16.  The value bias is folded into the
output bias on the host (attention weights sum to 1, so v's +bv contributes
exactly WO @ perm(WV_b) to every output row).  This removes every on-device
transpose, cast, and bias-broadcast from the critical path.

Per-core algorithm (matmuls bf16 with fp32 PSUM accumulation):
  - kT = wkT^T keyT (+bk via ACT bias), qT likewise           [c, s] layouts
  - per head pair: v_pair = valT^T wvT                        [s, c] natural
  - scores_h[s,q] = kT_h^T qT_h  (K=32 row-packed, 2 heads interleaved)
  - E = exp(scores) via ACT over [128, 1024] PSUM tiles (max|score| ~ 20,
    no max-subtraction needed), written straight to SBUF bf16
  - numerT_h[d,q] = v_h^T E  (PE, accumulated over s-tiles)
  - denom[q]: E tiles are tree-summed on DVE into a scratch tile (bf16
    tensor_tensor, 2x perf mode, read-only on E so it overlaps phase 2),
    finished by one ones-matmul per head, transposed to [q, 1], inverted.
    Streaming all 16 tiles through the PE instead would cost ~16x more PE
    time under the N-cycles-per-matmul cost model.
  - per pair: out[q,:] += (numerT_h^T WOT'_h) * (1/denom_h[q]) (+ bo at
    h==0); the 1/denom scale rides the per-partition scalar operand of
    scalar_tensor_tensor.

The main loop is software-pipelined over chunks i = 2*pg + qc with a
two-chunk phase-1 lead; each body runs phase2(i) (+ denominator tree on
DVE), den_pd(i), phase1(i+2), den_recip(i), the WO half for chunk i, and
the v-projection for the next head pair, so the PE stays fed through the
exp-paced score groups and the DVE/ACT drains.

kernel(**inputs) takes the FULL unsharded inputs and returns the full output.
"""

import numpy as np
from contextlib import ExitStack

import concourse.bass as bass
import concourse.bacc as bacc
import concourse.mybir as mybir
import concourse.tile as tile
from concourse.bass_utils import run_bass_kernel_spmd

P = 128
B, T1, T2, DM, DK, H = 4, 2048, 2048, 256, 32, 8
QSH = T2 // 2  # queries per core
N_CORES = 8

F32 = mybir.dt.float32
BF16 = mybir.dt.bfloat16
AF = mybir.ActivationFunctionType

ST = T1 // P        # 16 key/seq tiles
QT = QSH // P       # 8 query tiles per core
QC = 512            # query chunk (PSUM free dim)
NQC = QSH // QC     # 2 query chunks
TREE_STOP = 2       # DVE tree stops at this many tiles; PE ones-chain finishes


def _build_bass():
    nc = bacc.Bacc("TRN2", target_bir_lowering=False, debug=False)

    keyT_d = nc.dram_tensor("keyT_x", [DM, T1], BF16, kind="ExternalInput").ap()
    qryT_d = nc.dram_tensor("qryT_x", [DM, QSH], BF16, kind="ExternalInput").ap()
    valT_d = nc.dram_tensor("valT_x", [DM, T1], BF16, kind="ExternalInput").ap()
    wkT_d = nc.dram_tensor("wkT", [DM, DM], BF16, kind="ExternalInput").ap()
    wqT_d = nc.dram_tensor("wqT", [DM, DM], BF16, kind="ExternalInput").ap()
    wvT_d = nc.dram_tensor("wvT", [DM, H * DM], BF16, kind="ExternalInput").ap()
    woP_d = nc.dram_tensor("woP", [H * DM, DM], BF16, kind="ExternalInput").ap()
    wkb = nc.dram_tensor("wkb", [DM], F32, kind="ExternalInput").ap()
    wqb = nc.dram_tensor("wqb", [DM], F32, kind="ExternalInput").ap()
    wob = nc.dram_tensor("wob", [DM], F32, kind="ExternalInput").ap()
    out = nc.dram_tensor("out_y", [QSH, DM], F32, kind="ExternalOutput").ap()

    with tile.TileContext(nc, pool_alloc_mode="queue") as tc:
        with ExitStack() as ctx:
            _body(ctx, tc, keyT_d, qryT_d, valT_d, wkT_d, wqT_d, wvT_d, woP_d,
                  wkb, wqb, wob, out)
    nc.compile()
    return nc


def _body(ctx, tc, keyT_d, qryT_d, valT_d, wkT_d, wqT_d, wvT_d, woP_d,
          wkb, wqb, wob, out):
    nc = tc.nc
    consts = ctx.enter_context(tc.tile_pool(name="consts", bufs=1))
    main = ctx.enter_context(tc.tile_pool(name="main", bufs=1))
    # One PSUM pool for the whole kernel (8 banks via 4 tags):
    #   tag A: 2 banks x2  (scores)
    #   tag B: 1 bank  x2  (k/q projections, numerT accumulators)
    #   tag C: 1 bank  x1  (denominator + its transpose, WO even qt)
    #   tag D: 1 bank  x1  (v projection, WO odd qt)
    pP = ctx.enter_context(tc.tile_pool(name="pP", bufs=1, space="PSUM"))

    # rows 0 and 32 serve as 1x1 identities for the two per-head denominator
    # transposes (fmap and weight must share a base partition in codegen)
    ident_f1 = consts.tile([33, 1], F32)
    nc.vector.memset(ident_f1, 1.0)
    ones_bf = consts.tile([P, 1], BF16)
    nc.vector.memset(ones_bf, 1.0)
    warm = consts.tile([P, 512], BF16)
    nc.vector.memset(warm, 0.0)

    # PE p-state warmup: dummy matmuls keep the PE continuously busy through
    # the input-load window, so the projections and first scores run at the
    # full 2.4 GHz clock instead of the 1.2 GHz ramp state.
    pw = pP.tile([1, 512], F32, tag="C", name="warmup", bufs=1)
    for _ in range(7):
        nc.tensor.matmul(pw, ones_bf, warm, start=True, stop=True)

    def tp(ap):
        return ap.rearrange("(n p) d -> p n d", p=P)

    # ---- loads.  HWDGE (SP) carries the 7 bf16 tensors (<= 8 DMAHW lanes,
    # so no false serialization chains); Pool/SWDGE carries the fp32 biases.
    wkT = main.tile([P, 2, DM], BF16)     # [d%128, dt, c]
    nc.sync.dma_start(out=wkT, in_=tp(wkT_d))
    # keyT lands in chunks so the first k-projections (and with them the
    # first scores) start before the full tensor is resident
    keyT = main.tile([P, 2, T1], BF16)    # [d%128, dt, s]
    nc.sync.dma_start(out=keyT[:, :, 0:512], in_=tp(keyT_d)[:, :, 0:512])
    wqT = main.tile([P, 2, DM], BF16)
    nc.sync.dma_start(out=wqT, in_=tp(wqT_d))
    qryT = main.tile([P, 2, QSH], BF16)
    nc.sync.dma_start(out=qryT[:, :, 0:512], in_=tp(qryT_d)[:, :, 0:512])
    nc.sync.dma_start(out=keyT[:, :, 512:T1], in_=tp(keyT_d)[:, :, 512:T1])
    nc.sync.dma_start(out=qryT[:, :, 512:QSH], in_=tp(qryT_d)[:, :, 512:QSH])
    valT = main.tile([P, 2, T1], BF16)
    nc.sync.dma_start(out=valT, in_=tp(valT_d))
    wvT = main.tile([P, 2, H * DM], BF16)  # [dm%128, dmt, c]
    nc.sync.dma_start(out=wvT, in_=tp(wvT_d))
    woTp = main.tile([P, 16, DM], BF16)   # [c'=h*256+d, cout]
    nc.sync.dma_start(out=woTp, in_=tp(woP_d))

    # biases; wk_b[p, t] = wkb[t*128+p] so kT tile ct gets bias wk_b[:, ct]
    wk_b = consts.tile([P, 2], F32)
    nc.gpsimd.dma_start(out=wk_b, in_=wkb.rearrange("(t p) -> p t", p=P))
    wq_b = consts.tile([P, 2], F32)
    nc.gpsimd.dma_start(out=wq_b, in_=wqb.rearrange("(t p) -> p t", p=P))
    # effective output bias (host-folded: WO_b + WO_w @ WV_b, since the
    # attention weights sum to one the value bias lands here exactly)
    wob_bc = consts.tile([P, DM], F32)
    nc.gpsimd.dma_start(
        out=wob_bc,
        in_=bass.AP(tensor=wob.tensor, offset=wob.offset, ap=[[0, P], [1, DM]]),
    )

    kT = main.tile([P, 2, T1], BF16)         # [c, s]
    qT = main.tile([P, 2, QSH], BF16)        # [c, q]
    # unnormalized numerators, two rotating head-pair slots [pg%2, 2*hh+dh, q]
    numerT = main.tile([P, 2, 4, QSH], BF16)
    recip = main.tile([P, H, QT], F32)       # [q%128, h, q//128] = 1/denom
    acc = main.tile([P, QT, DM], F32)        # output accumulator [q, cout]

    # ---- k/q projections: kT[c, s] = sum_d wkT[d, c] keyT[d, s]  (+bias).
    # Emission order is earliest-consumer-first: the first scores need kT sc0
    # and qT sc0, so those four projections (and their ACT bias copies) come
    # before the remaining key chunks.
    def kqproj(wT, srcT, dstT, bias, sc, pfx):
        for ct in range(2):
            pp = pP.tile([P, 512], F32, tag="B", name=f"{pfx}{ct}_{sc}", bufs=2)
            for dt in range(2):
                nc.tensor.matmul(pp, wT[:, dt, ct * P:(ct + 1) * P],
                                 srcT[:, dt, sc * 512:(sc + 1) * 512],
                                 start=(dt == 0), stop=(dt == 1))
            nc.scalar.activation(out=dstT[:, ct, sc * 512:(sc + 1) * 512],
                                 in_=pp, func=AF.Identity,
                                 bias=bias[:, ct:ct + 1])

    kqproj(wkT, keyT, kT, wk_b, 0, "ppk")
    kqproj(wqT, qryT, qT, wq_b, 0, "ppq")
    kqproj(wkT, keyT, kT, wk_b, 1, "ppk")
    kqproj(wqT, qryT, qT, wq_b, 1, "ppq")
    for sc in range(2, T1 // 512):
        kqproj(wkT, keyT, kT, wk_b, sc, "ppk")

    # ---------------- main loop: attention per head pair --------------------
    # Software pipeline over chunks i = 2*pg + qc: the denominator's PE tail
    # (ones-matmuls, transpose, reciprocal) for chunk i is emitted inside
    # chunk i+1, after its scores, so the PE never waits on the DVE tree;
    # WO(pg) is emitted inside chunk 2*pg+2.
    mult, add = mybir.AluOpType.mult, mybir.AluOpType.add
    with ExitStack() as sm:
        sE = sm.enter_context(tc.tile_pool(name="sE", bufs=4))
        sv = sm.enter_context(tc.tile_pool(name="sv", bufs=2))
        ssm = sm.enter_context(tc.tile_pool(name="ssm", bufs=2))

        v_pairs = {}

        def vproj_unit(pg, st):
            # alternate PSUM tags C/D for double buffering (each is one
            # bank; together they pipeline the PE drain)
            v_pair = v_pairs[pg]
            pvt = pP.tile([P, 512], F32, tag=("C", "D")[st % 2],
                          name=f"pv{pg}_{st}", bufs=1)
            for dt in range(2):
                nc.tensor.matmul(pvt, valT[:, dt, st * P:(st + 1) * P],
                                 wvT[:, dt, pg * 512:(pg + 1) * 512],
                                 start=(dt == 0), stop=(dt == 1))
            nc.vector.tensor_copy(out=v_pair[:, st, :], in_=pvt)

        def vproj_units(pg):
            v_pairs[pg] = sv.tile([P, ST, 512], BF16, tag="vp", name=f"vp{pg}")
            return [lambda pg=pg, st=st: vproj_unit(pg, st) for st in range(ST)]

        def phase1(pg, qc, Es, extras=()):
            # scores + exp.  scores_h[s, q] = kT_h^T qT_h.
            # The score groups are exp-paced (the ACT is slower than the four
            # matmuls), so `extras` -- small independent PE work items -- are
            # interleaved between groups to fill the PSUM-recycle waits.
            ex = list(extras)
            h0 = 2 * pg
            for sp in range(ST // 2):
                pss = [pP.tile([P, 2, QC], F32, tag="A",
                               name=f"sc{h0 + i}_{qc}_{sp}", bufs=2)
                       for i in range(2)]
                # interleave the two heads so consecutive matmuls hit
                # different 32-row strips of the PE array (row packing)
                for i in range(2):
                    st = 2 * sp + i
                    for hh in range(2):
                        h = h0 + hh
                        base, ctile = 32 * (h % 4), h // 4
                        nc.tensor.matmul(
                            pss[hh][:, i, :],
                            kT[base:base + 32, ctile, st * P:(st + 1) * P],
                            qT[base:base + 32, ctile, qc * QC:(qc + 1) * QC],
                            start=True, stop=True, tile_position=(base, 0))
                for hh in range(2):
                    nc.scalar.activation(out=Es[hh][:, 2 * sp:2 * sp + 2, :],
                                         in_=pss[hh], func=AF.Exp)
                if sp >= 1 and ex:
                    ex.pop(0)()
            for f in ex:
                f()

        def tree_head(ts, hh, Es):
            # denominator tree-sum for one head on DVE into scratch (bf16
            # tensor_tensor, 2x perf mode).  Read-only on the E tiles, so it
            # overlaps phase 2 on the PE.
            nc.vector.tensor_add(ts[:, hh], Es[hh][:, 0:ST // 2, :],
                                 Es[hh][:, ST // 2:ST, :])
            n = ST // 2
            while n > TREE_STOP:
                n //= 2
                nc.vector.tensor_add(ts[:, hh, 0:n, :], ts[:, hh, 0:n, :],
                                     ts[:, hh, n:2 * n, :])

        def phase2(pg, qc, Es):
            # numerT_h[d, q] = v_h^T E_h.  DVE-queue order per chunk is
            # tree(h0), dh0 copies, tree(h1), dh1 copies: the tree halves
            # overlap the matmul passes and finish before den_pd needs them.
            # dh1's accumulators use PSUM tag A (idle between score phases) so
            # its matmuls never wait on dh0's PSUM drains.
            h0 = 2 * pg
            v_pair = v_pairs[pg]
            ts = ssm.tile([P, 2, ST // 2, QC], BF16, tag="ts",
                          name=f"ts{pg}_{qc}", bufs=1)
            tree_head(ts, 0, Es)
            for dh in range(2):
                pas = [pP.tile([P, QC], F32, tag="B",
                               name=f"pa{h0 + i}_{qc}_{dh}", bufs=2)
                       for i in range(2)]
                for st in range(ST):
                    for hh in range(2):
                        nc.tensor.matmul(
                            pas[hh],
                            v_pair[:, st, hh * 256 + dh * P: hh * 256 + (dh + 1) * P],
                            Es[hh][:, st, :],
                            start=(st == 0), stop=(st == ST - 1))
                for hh in range(2):
                    nc.vector.tensor_copy(
                        out=numerT[:, pg % 2, 2 * hh + dh, qc * QC:(qc + 1) * QC],
                        in_=pas[hh])
                if dh == 0:
                    tree_head(ts, 1, Es)
            return ts

        def den_pd(pg, qc, ts):
            # finish the denominator sum on the PE; stage to SBUF for the
            # transpose (single copy grabs both head rows 0 and 32)
            pd = pP.tile([P, QC], F32, tag="C", name=f"pd{pg}_{qc}", bufs=1)
            for hh in range(2):
                for t in range(TREE_STOP):
                    nc.tensor.matmul(
                        pd[32 * hh:32 * hh + 1, :], ones_bf, ts[:, hh, t, :],
                        start=(t == 0), stop=(t == TREE_STOP - 1),
                        tile_position=(0, 32 * hh), skip_group_check=True)
            dsb = ssm.tile([33, QC], F32, tag="dsb", name=f"dsb{pg}_{qc}")
            nc.scalar.copy(out=dsb, in_=pd[0:33, :])
            return dsb

        def den_recip(pg, qc, dsb):
            # transpose [1, q] -> [q%128, q//128] and invert
            pdt = pP.tile([P, 2, QC // P], F32, tag="C",
                          name=f"pdt{pg}_{qc}", bufs=1)
            for hh in range(2):
                h = 2 * pg + hh
                for j in range(QC // P):
                    nc.tensor.transpose(
                        pdt[:, hh, j:j + 1],
                        dsb[32 * hh:32 * hh + 1, j * P:(j + 1) * P],
                        ident_f1[32 * hh:32 * hh + 1, :],
                        tile_position=(32 * hh, 0))
                nc.vector.reciprocal(
                    out=recip[:, h, qc * (QC // P):(qc + 1) * (QC // P)],
                    in_=pdt[:, hh, :])

        def wo_unit(pg, qt, hh):
            # acc[q, :] += (numerT_h^T WOT'_h) * recip_h[q]  (+= bias at h==0)
            h = 2 * pg + hh
            po = pP.tile([P, DM], F32, tag=("C", "D")[qt % 2],
                         name=f"po{qt}_{h}", bufs=1)
            for dh in range(2):
                nc.tensor.matmul(po,
                                 numerT[:, pg % 2, 2 * hh + dh,
                                        qt * P:(qt + 1) * P],
                                 woTp[:, 2 * h + dh, :],
                                 start=(dh == 0), stop=(dh == 1))
            nc.vector.scalar_tensor_tensor(
                out=acc[:, qt, :], in0=po, scalar=recip[:, h, qt:qt + 1],
                in1=(wob_bc if h == 0 else acc[:, qt, :]),
                op0=mult, op1=add)

        def wo_units(pg, qts):
            return [lambda pg=pg, qt=qt, hh=hh: wo_unit(pg, qt, hh)
                    for qt in qts for hh in range(2)]

        out_r = out.rearrange("(n p) d -> p n d", p=P)

        def store(qts):
            for qt in qts:
                nc.sync.dma_start(out=out_r[:, qt, :], in_=acc[:, qt, :])

        # Chunk pipeline with a two-chunk phase-1 lead: scores for chunks 0
        # and 1 are emitted up front (filling the PE while val/wv still
        # load), and each body then runs phase2(i) + denominator(i) +
        # phase1(i+2) + recip/WO(i), so the PE never waits on the DVE tree
        # or the denominator staging.  WO is emitted per query-half as soon
        # as the matching chunk's recip lands.
        last = H // 2 - 1
        chunks = [(pg, qc) for pg in range(H // 2) for qc in range(NQC)]
        all_Es = {}

        def ph1(i, extras=()):
            pg, qc = chunks[i]
            Es = [sE.tile([P, ST, QC], BF16, tag="E", name=f"E{2 * pg + j}_{qc}")
                  for j in range(2)]
            all_Es[i] = Es
            phase1(pg, qc, Es, extras)

        ph1(0)
        ph1(1)
        for u in vproj_units(0):
            u()
        for i, (pg, qc) in enumerate(chunks):
            Es = all_Es.pop(i)
            ts = phase2(pg, qc, Es)
            dsb = den_pd(pg, qc, ts)
            half = range(QT // 2) if qc == 0 else range(QT // 2, QT)
            units = [lambda: den_recip(pg, qc, dsb)]
            units += wo_units(pg, half)
            if qc == 1 and pg + 1 <= last:
                units += vproj_units(pg + 1)
            if i + 2 < len(chunks):
                ph1(i + 2, units)
            else:
                for u in units:
                    u()
            if pg == last:
                store(half)


_NC_CACHE = None


def _get_nc():
    global _NC_CACHE
    if _NC_CACHE is None:
        _NC_CACHE = _build_bass()
    return _NC_CACHE


def _make_in_maps(inputs):
    from ml_dtypes import bfloat16

    f32 = lambda x: np.ascontiguousarray(np.asarray(x, dtype=np.float32))
    bft = lambda x: np.ascontiguousarray(
        np.asarray(x, dtype=np.float32).T.astype(bfloat16))
    # WO column-permuted head-outer: woP[h*256+d, o] = WO[o, d*8+h]
    wo = np.asarray(inputs["WO_w"], dtype=np.float32)
    woP = np.ascontiguousarray(
        wo.reshape(DM, DM, H).transpose(2, 1, 0).reshape(H * DM, DM).astype(bfloat16))
    shared = {
        "wkT": bft(inputs["WK_w"]), "wkb": f32(inputs["WK_b"]),
        "wqT": bft(inputs["WQ_w"]), "wqb": f32(inputs["WQ_b"]),
        "wvT": bft(inputs["WV_w"]),
        "woP": woP,
        # value bias folded through WO (attention weights sum to 1)
        # WV_b is indexed h*256+d; WO columns are d*8+h -> permute first
        "wob": f32(np.asarray(inputs["WO_b"], dtype=np.float32)
                   + wo @ np.asarray(inputs["WV_b"], dtype=np.float32)
                          .reshape(H, DM).T.ravel()),
    }
    key_in = np.asarray(inputs["key_input"], dtype=np.float32)
    qry_in = np.asarray(inputs["query_input"], dtype=np.float32)
    val_in = np.asarray(inputs["value_input"], dtype=np.float32)
    in_maps = []
    for c in range(N_CORES):
        b, qs = c // 2, c % 2
        in_maps.append(dict(
            shared,
            keyT_x=bft(key_in[b]),
            qryT_x=bft(qry_in[b, qs * QSH:(qs + 1) * QSH]),
            valT_x=bft(val_in[b]),
        ))
    return in_maps


def _assemble(results):
    out = np.empty((B, T2, DM), dtype=np.float32)
    for c in range(N_CORES):
        b, qs = c // 2, c % 2
        out[b, qs * QSH:(qs + 1) * QSH] = results[c]["out_y"]
    return out


def run_spmd(inputs, **kwargs):
    """Run the kernel on all 8 cores; kwargs forwarded (e.g. trace=True)."""
    nc = _get_nc()
    res = run_bass_kernel_spmd(nc, _make_in_maps(inputs),
                               core_ids=list(range(N_CORES)), **kwargs)
    return res


def kernel(**inputs):
    res = run_spmd(inputs)
    return _assemble(res.results)
